# revision 35
# baseline (speedup 1.0000x reference)
"""Builder + host-side prep for nn_Attention distributed kernel.

Strategy: pure data-parallel sharding over (batch, query-row-half).
Core c handles batch b=c//2, query rows i0=(c%2)*512 .. i0+512.
No collectives needed: K/V are computed per-core from the full sequence
of its batch; each core's output rows are disjoint.

All attention math is done in "transposed score" layout S^T[j, i]
(j = key position on partitions, i = query on free axis) so the
probability matrix lands PV-ready without any on-chip transposes:
  - q^T, k^T projections: [e, n] layout from x^T (host pre-transposed)
  - S^T  = matmul(lhsT=k^T slice [d,j], rhs=q^T slice [d,i])
  - P^T  = exp(S^T) * exp(bias^T)   (host precomputes exp of bias;
    multiply on DVE).
  - out^T_h [33, i] = matmul(lhsT=v_aug [j, 33], rhs=P^T [j, i]) summed
    over j-chunks; column 32 of v_aug is ones -> row 32 = softmax denom.
    The two heads of a pair share one PSUM bank (partition offsets
    0/64) which lets their PV matmuls co-execute in the PE array.
  - PV batches execute two rounds behind QK batches (dep-edge stagger)
    so the exp->multiply chain is never on the QK critical path and the
    scalar engine streams exps back-to-back.
  - gating sigmoid via tanh (same ACT table set as exp); e-chunk 0's
    normalization broadcast via DMA-bounce mid-stream; chunk 1 via an
    indicator matmul at the tail (PE idle there), with the last head
    pair's PSUM evacuated on the scalar engine and the y-projection's
    first contraction half hoisted ahead of the normalization chain.
"""

import sys

if "/opt/trn_rl_repo" not in sys.path:
    sys.path.insert(0, "/opt/trn_rl_repo")

from contextlib import ExitStack

import ml_dtypes
import numpy as np

import concourse.bass as bass
import concourse.tile as tile
from concourse import bacc, mybir
from concourse.bass import ts

P = 128
B, N, DQ = 4, 1024, 256
H, D = 8, 32
NI = 512  # query rows per core
NCORES = 8

BF16 = mybir.dt.bfloat16
F32 = mybir.dt.float32
NPBF16 = ml_dtypes.bfloat16

EXPF = mybir.ActivationFunctionType.Exp
TANHF = mybir.ActivationFunctionType.Tanh
DIV = mybir.AluOpType.divide
COPYF = mybir.ActivationFunctionType.Copy



def build_nc():
    nc = bacc.Bacc(None, target_bir_lowering=False, debug=False)

    # DRAM parameters (identical graph on all 8 cores; shards differ)
    # exp(bias^T + mask), pre-tiled so each (h, jp) DMA is contiguous:
    # eb[h, jp, p, t*NI + i] = exp(bias^T)[h, (jp*2+t)*128 + p, i]
    eb_d = nc.declare_dram_parameter("eb", [H, 4, P, 2 * NI], BF16, False)
    # all bf16 weights + x packed per-partition: one DMA loads everything
    # layout per p: wq(2*256) wk wv wg wo | xt(2*1024) xqt(2*512)
    wpack_d = nc.declare_dram_parameter("wpack", [P, 5632], BF16, False)
    hbg_d = nc.declare_dram_parameter("hbg", [P, 2], F32, False)   # bg/2 as [p, chunk]
    bob_d = nc.declare_dram_parameter("bob", [P, 2], F32, False)   # bo as [p, c_chunk]
    ind_d = nc.declare_dram_parameter("ind", [8, 256], BF16, False)  # (e//32 == h)
    y_d = nc.declare_dram_parameter("out", [2, P, NI], BF16, True)  # y^T chunked
    dnrec_d = nc.dram_tensor("dnrec", [8, NI], F32)                # internal scratch

    from concourse.tile_rust import add_dep_helper

    with tile.TileContext(nc) as tc, ExitStack() as ctx:
        singles = ctx.enter_context(tc.tile_pool(name="singles", bufs=1))
        spsum = ctx.enter_context(tc.tile_pool(name="spsum", bufs=3, space="PSUM"))
        opsum = ctx.enter_context(tc.tile_pool(name="opsum", bufs=2, space="PSUM"))
        ebuf = ctx.enter_context(tc.tile_pool(name="ebuf", bufs=6))
        esb = ctx.enter_context(tc.tile_pool(name="esb", bufs=4))
        ptb = ctx.enter_context(tc.tile_pool(name="ptb", bufs=4))
        ostb = ctx.enter_context(tc.tile_pool(name="ostb", bufs=2))

        _sc = [0]

        def sslot():
            _sc[0] += 1
            return spsum.tile([P, 1024], F32, tag="s_ps", name=f"s_ps{_sc[0]}")

        # ---- load constants / weights (two packed DMAs on two queues) ----
        wpA = singles.tile([P, 4096], BF16, name="wpA")  # wq wk xt xqt
        wpB = singles.tile([P, 1536], BF16, name="wpB")  # wv wg wo
        nc.sync.dma_start(out=wpA, in_=wpack_d[:, 0:4096])
        nc.scalar.dma_start(out=wpB, in_=wpack_d[:, 4096:5632])
        wq_sb = wpA[:, 0:512].rearrange("p (k e) -> p k e", k=2)
        wk_sb = wpA[:, 512:1024].rearrange("p (k e) -> p k e", k=2)
        xt_sb = wpA[:, 1024:3072].rearrange("p (k n) -> p k n", k=2)
        xqt_sb = wpA[:, 3072:4096].rearrange("p (k i) -> p k i", k=2)
        wv_sb = wpB[:, 0:512].rearrange("p (k e) -> p k e", k=2)
        wg_sb = wpB[:, 512:1024].rearrange("p (k e) -> p k e", k=2)
        wo_sb = wpB[:, 1024:1536].rearrange("p (k e) -> p k e", k=2)
        hbg_sb = singles.tile([P, 2], F32)
        bob_sb = singles.tile([P, 2], F32)
        ind_sb = singles.tile([8, 256], BF16)
        nc.sync.dma_start(out=hbg_sb, in_=hbg_d[:])
        nc.sync.dma_start(out=bob_sb, in_=bob_d[:])
        nc.sync.dma_start(out=ind_sb, in_=ind_d[:])

        # ACT table preload: dummy Exp at t=0 so the 2.7us table load
        # overlaps the weight DMA instead of stalling the first real use
        warm = singles.tile([P, 8], F32)
        nc.vector.memset(warm, 1.0)
        nc.scalar.activation(out=warm, in_=warm, func=EXPF)

        # ---- projection targets ----
        kt_sb = [singles.tile([P, N], BF16, name=f"kt{m}") for m in range(2)]
        qt_sb = [singles.tile([P, NI], BF16, name=f"qt{m}") for m in range(2)]
        vaug_sb = [singles.tile([P, 2, H, 33], BF16, name=f"vaug{j}")
                   for j in range(4)]
        sig_sb = singles.tile([P, 2, NI], F32)    # sigmoid(gates)^T [e, i]
        ogt_un = singles.tile([P, 2, NI], F32)    # unnormalized gated^T staging

        def v_round(jtp):
            ps = sslot()
            f = l = None
            for u in range(2):
                jt = jtp * 2 + u
                for kc in range(2):
                    l = nc.tensor.matmul(
                        ps[:, u * 512 : u * 512 + 256],
                        lhsT=xt_sb[:, kc, ts(jt, P)], rhs=wv_sb[:, kc, :],
                        start=(kc == 0), stop=(kc == 1),
                    )
                    f = f or l
            for u in range(2):
                nc.vector.tensor_copy(
                    out=vaug_sb[jtp][:, u, :, 0:32],
                    in_=ps[:, u * 512 : u * 512 + 256].rearrange(
                        "p (h d) -> p h d", h=H),
                )
            return f, l

        def qk_round(m, part):
            # part 0: q chunk m + k chunk m first half; part 1: k second half
            ps = sslot()
            f = l = None
            if part == 0:
                for kc in range(2):
                    l = nc.tensor.matmul(
                        ps[:, :NI], lhsT=wq_sb[:, kc, ts(m, P)],
                        rhs=xqt_sb[:, kc, :], start=(kc == 0), stop=(kc == 1))
                    f = f or l
                for kc in range(2):
                    l = nc.tensor.matmul(
                        ps[:, NI:], lhsT=wk_sb[:, kc, ts(m, P)],
                        rhs=xt_sb[:, kc, :512], start=(kc == 0), stop=(kc == 1))
                nc.vector.tensor_copy(out=qt_sb[m], in_=ps[:, :NI])
                nc.vector.tensor_copy(out=kt_sb[m][:, 0:512], in_=ps[:, NI:])
            else:
                for kc in range(2):
                    l = nc.tensor.matmul(
                        ps[:, :NI], lhsT=wk_sb[:, kc, ts(m, P)],
                        rhs=xt_sb[:, kc, 512:], start=(kc == 0), stop=(kc == 1))
                    f = f or l
                nc.vector.tensor_copy(out=kt_sb[m][:, 512:], in_=ps[:, :NI])
            return f, l

        def g_round():
            # gates^T: sigmoid via tanh: sig = 0.5*tanh((g+bg)/2) + 0.5
            ps = sslot()
            f = l = None
            for m in range(2):
                for kc in range(2):
                    l = nc.tensor.matmul(
                        ps[:, ts(m, NI)], lhsT=wg_sb[:, kc, ts(m, P)],
                        rhs=xqt_sb[:, kc, :], start=(kc == 0), stop=(kc == 1))
                    f = f or l
            for m in range(2):
                nc.scalar.activation(out=sig_sb[:, m, :], in_=ps[:, ts(m, NI)],
                                     func=TANHF, bias=hbg_sb[:, m : m + 1],
                                     scale=0.5)
            nc.vector.tensor_scalar(out=sig_sb, in0=sig_sb, scalar1=0.5,
                                    scalar2=0.5, op0=mybir.AluOpType.mult,
                                    op1=mybir.AluOpType.add)
            return f, l

        # pre-stream: q/k chunk 0 + first v pair (everything else is injected
        # into the stream between QK and PV batches)
        qk_round(0, 0)
        for j in range(4):
            nc.vector.memset(vaug_sb[j][:, :, :, 32:33], 1.0)

        # injected work, keyed by stream round; PV emission lags two rounds
        # so each vaug tile only needs to be emitted one round before its
        # first PV consumer
        inject = {
            0: [lambda: qk_round(0, 1)],
            1: [lambda: v_round(0)],
            2: [lambda: v_round(1)],
            3: [lambda: v_round(2)],
            4: [lambda: v_round(3)],
            5: [lambda: qk_round(1, 0)],
            6: [lambda: qk_round(1, 1)],
            7: [g_round],
        }

        # ---- attention stream state ----
        dn8 = singles.tile([8, NI], F32)        # per-head denominators
        nc.vector.memset(dn8, 1.0)              # rows read before all written
        sigf = singles.tile([P, 2, NI], F32)    # sig * (1/denom broadcast)
        ogt = singles.tile([P, 2, NI], BF16)    # normalized gated out^T

        qk_insts, pv_insts, inj_insts = [], [], []

        def chunk_bounce(hc, rows, dmae):
            # reciprocal of denominators; broadcast each head's row to its
            # 32 partitions via a DRAM bounce with a stride-0 AP (no compute
            # engine in the path -> no PE queue blocking).  `rows` selects
            # which 32-partition groups of chunk hc to produce.
            rec = ostb.tile([8, NI], F32, tag="rec", name=f"rec{hc}_{rows[0]}")
            nc.vector.reciprocal_approx_fast(out=rec, in_=dn8)
            dmae.dma_start(out=dnrec_d[:], in_=rec)
            rb = ostb.tile([P, NI], F32, tag="rb", name=f"rb{hc}_{rows[0]}")
            for g in rows:
                sl = dnrec_d[hc * 4 + g : hc * 4 + g + 1, :]
                bcast_ap = bass.AP(tensor=sl.tensor, offset=sl.offset,
                                   ap=[[0, 32], list(sl.ap[1])])
                dmae.dma_start(out=rb[g * 32 : (g + 1) * 32, :], in_=bcast_ap)
            lo, hi = rows[0] * 32, (rows[-1] + 1) * 32
            nc.vector.tensor_mul(out=sigf[lo:hi, hc, :],
                                 in0=sig_sb[lo:hi, hc, :], in1=rb[lo:hi, :])
            nc.vector.tensor_mul(out=ogt[lo:hi, hc, :],
                                 in0=ogt_un[lo:hi, hc, :],
                                 in1=sigf[lo:hi, hc, :])

        o_tiles, pt_tiles = {}, {}

        def emit_pv(r):
            hp, jp = divmod(r, 4)
            if hp not in o_tiles:
                o_tiles[hp] = opsum.tile([P, NI], F32, tag="o",
                                         name=f"o_t{hp}")
            o_t = o_tiles[hp]
            pt = pt_tiles.pop(r)
            pf = pl = None
            for t in range(2):
                jc = jp * 2 + t
                for u, h in enumerate((2 * hp, 2 * hp + 1)):
                    pl = nc.tensor.matmul(
                        o_t[64 * u : 64 * u + 33, :],
                        lhsT=vaug_sb[jc // 2][:, jc % 2, h, :],
                        rhs=pt[h][:, t, :],
                        start=(jc == 0), stop=(jc == 7),
                        skip_group_check=True,
                    )
                    pf = pf or pl
            return pf, pl

        def emit_dumps(hp):
            o_t = o_tiles.pop(hp)
            for u, h in enumerate((2 * hp, 2 * hp + 1)):
                hq, hc = h % 4, h // 4
                ost = ostb.tile([33, NI], F32)
                if hp == 3:
                    # scalar engine is idle after the last exp
                    nc.scalar.activation(out=ost, in_=o_t[64 * u : 64 * u + 33, :],
                                         func=COPYF)
                    ea = nc.sync if h % 2 else nc.scalar
                    eb_ = nc.scalar if h % 2 else nc.sync
                else:
                    nc.vector.tensor_copy(out=ost,
                                          in_=o_t[64 * u : 64 * u + 33, :])
                    ea = eb_ = nc.gpsimd
                ea.dma_start(
                    out=ogt_un[hq * 32 : (hq + 1) * 32, hc, :], in_=ost[0:32, :]
                )
                eb_.dma_start(out=dn8[h : h + 1, :], in_=ost[32:33, :])

        for rnd in range(16):
            hpair, jp = divmod(rnd, 4)
            heads = (2 * hpair, 2 * hpair + 1)
            ebt, s_ps, pt = {}, {}, {}
            for h in heads:
                ebt[h] = ebuf.tile([P, 2, NI], BF16, tag="ebt",
                                   name=f"ebt{h}_{jp}")
                dq = nc.sync if h % 2 == 0 else nc.gpsimd
                dq.dma_start(out=ebt[h], in_=eb_d[h, jp])
            # interleaved QK batch: consecutive MMs hit different PE row
            # groups -> concurrent in the array
            for h in heads:
                s_ps[h] = sslot()
            qf = ql = None
            for t in range(2):
                jc = jp * 2 + t
                for h in heads:
                    hq, hc = h % 4, h // 4
                    ql = nc.tensor.matmul(
                        s_ps[h][:, ts(t, 512)],
                        lhsT=kt_sb[hc][hq * 32 : (hq + 1) * 32, ts(jc, P)],
                        rhs=qt_sb[hc][hq * 32 : (hq + 1) * 32, :],
                        start=True, stop=True,
                        tile_position=(hq * 32, 0),
                        skip_group_check=True,
                    )
                    qf = qf or ql
            qk_insts.append((qf, ql))
            # injected K=128 projection work sits between the QK and PV
            # batches (same array mode as PV -> no extra mode switches)
            ij = []
            for fn in inject.get(rnd, []):
                ij.append(fn())
            inj_insts.append(ij)
            for h in heads:
                pt[h] = ptb.tile([P, 2, NI], BF16, tag="pt",
                                 name=f"pt{h}_{rnd}")
                es = esb.tile([P, 2, NI], BF16, tag="es", name=f"es{h}_{jp}")
                nc.scalar.activation(
                    out=es, in_=s_ps[h][:].rearrange("p (t i) -> p t i", t=2),
                    func=EXPF,
                )
                nc.vector.tensor_mul(out=pt[h], in0=es, in1=ebt[h])
            pt_tiles[rnd] = pt
            if rnd >= 2:
                pv_insts.append(emit_pv(rnd - 2))
            if rnd % 4 == 1 and rnd >= 5:
                emit_dumps(rnd // 4 - 1)
            if rnd == 9:
                chunk_bounce(0, [0, 1, 2, 3], nc.gpsimd)
        pv_insts.append(emit_pv(14))
        pv_insts.append(emit_pv(15))
        emit_dumps(3)

        # PE batch order (the PE runs throttled when K=32 tile-position MMs
        # interleave with K=128 ones): [QK][inj][PV(r-2)] per round, staggered
        # two deep so exp/multiply never gate the QK stream:
        for r in range(len(qk_insts)):
            if r >= 3:
                add_dep_helper(qk_insts[r][0].ins, pv_insts[r - 3][1].ins,
                               sync=False, reason="qk(r) after pv(r-3)")
            for f, l in inj_insts[r]:
                add_dep_helper(f.ins, qk_insts[r][1].ins, sync=False,
                               reason="inj after qk batch")
            if r + 2 < len(qk_insts):
                prev = (inj_insts[r + 2][-1][1] if inj_insts[r + 2]
                        else qk_insts[r + 2][1])
                add_dep_helper(pv_insts[r][0].ins, prev.ins, sync=False,
                               reason="pv(r) after qk/inj(r+2)")

        # ---- tail: y-projection ec=0 halves run as soon as the PE is
        # free (ogt chunk 0 completed mid-stream); then the chunk-1
        # normalization (indicator-matmul broadcast, heads 4-7 in one go),
        # then the closing ec=1 matmuls + scalar-engine biased copies ----
        yps = sslot()
        for cc in range(2):
            nc.tensor.matmul(
                yps[:, ts(cc, NI)], lhsT=wo_sb[:, 0, ts(cc, P)],
                rhs=ogt[:, 0, :], start=True, stop=False,
                skip_group_check=True,
            )
        rec = ostb.tile([8, NI], F32, tag="rec", name="rec1")
        nc.vector.reciprocal_approx_fast(out=rec, in_=dn8)
        recb = ostb.tile([8, NI], BF16, tag="recb", name="recb1")
        nc.vector.tensor_copy(out=recb, in_=rec)
        bps = opsum.tile([P, NI], F32, tag="o", name="bps1")
        nc.tensor.matmul(bps[:, :NI], lhsT=ind_sb[:, ts(1, P)],
                         rhs=recb, start=True, stop=True)
        nc.vector.tensor_mul(out=sigf[:, 1, :], in0=sig_sb[:, 1, :],
                             in1=bps[:, :NI])
        nc.vector.tensor_mul(out=ogt[:, 1, :], in0=ogt_un[:, 1, :],
                             in1=sigf[:, 1, :])
        for cc in range(2):
            nc.tensor.matmul(
                yps[:, ts(cc, NI)], lhsT=wo_sb[:, 1, ts(cc, P)],
                rhs=ogt[:, 1, :], start=False, stop=True,
                skip_group_check=True,
            )
            ysb = ostb.tile([P, NI], BF16, tag="ysb", name=f"ysb{cc}")
            nc.vector.tensor_scalar_add(out=ysb, in0=yps[:, ts(cc, NI)],
                                        scalar1=bob_sb[:, cc : cc + 1])
            dq = nc.sync if cc == 0 else nc.scalar
            dq.dma_start(out=y_d[cc], in_=ysb)

    nc.compile()
    return nc


def prep_core_inputs(core, x, mask, attn_bias, Wq, Wkv, Wo, bo, Wg, bg):
    """Host-side shard + layout prep for one core. All numpy."""
    b, ih = core // 2, core % 2
    i0 = ih * NI
    scale = D ** -0.5

    xt = np.ascontiguousarray(x[b].T)  # [256, N]
    amask = np.where(mask[b] > 0, 0.0, -200.0).astype(np.float32)  # [N] over j
    bt = attn_bias[b, :, i0 : i0 + NI, :].transpose(0, 2, 1)  # [H, j, i]
    bt = bt + amask[None, :, None]
    eb = np.exp(bt).astype(NPBF16)
    # pre-tile for contiguous per-(h, jp) DMAs: [H, 4, 128, 2*NI]
    eb = np.ascontiguousarray(
        eb.reshape(H, 4, 2, P, NI).transpose(0, 1, 3, 2, 4).reshape(H, 4, P, 2 * NI)
    )

    def chunk(wT):  # [256, X] -> [2, 128, X] bf16
        return np.ascontiguousarray(wT.reshape(2, P, -1)).astype(NPBF16)

    ind = np.zeros((8, 256), np.float32)
    for h in range(H):
        ind[h, h * 32 : (h + 1) * 32] = 1.0


    def flat(wT):  # [256, X] -> [128, 2*X] per-partition pack
        c = chunk(wT)  # [2, 128, X]
        return c.transpose(1, 0, 2).reshape(P, -1)

    wpack = np.concatenate(
        [flat((Wq * scale).T), flat(Wkv[:256].T), flat(xt),
         flat(xt[:, i0 : i0 + NI]), flat(Wkv[256:].T), flat(Wg.T),
         flat(Wo.T)], axis=1)
    return {
        "wpack": np.ascontiguousarray(wpack),
        "eb": np.ascontiguousarray(eb),
        "hbg": np.ascontiguousarray((bg * 0.5).reshape(2, P).T).astype(np.float32),
        "bob": np.ascontiguousarray(bo.astype(np.float32).reshape(2, P).T),
        "ind": ind.astype(NPBF16),
    }


def prep_all_inputs(**inputs):
    inputs = {k: np.asarray(v, dtype=np.float32) for k, v in inputs.items()}
    return [prep_core_inputs(c, **inputs) for c in range(NCORES)]


def gather_outputs(results):
    """results: per-core dicts with 'out' = y^T chunked [2, P, NI] -> [B, N, DQ]."""
    y = np.zeros((B, N, DQ), np.float32)
    for c in range(NCORES):
        b, ih = c // 2, c % 2
        yt = np.asarray(results[c]["out"]).astype(np.float32).reshape(DQ, NI)
        y[b, ih * NI : (ih + 1) * NI, :] = yt.T
    return y


_NC_CACHE = None


def _get_nc():
    global _NC_CACHE
    if _NC_CACHE is None:
        _NC_CACHE = build_nc()
    return _NC_CACHE


def kernel(**inputs):
    """Full (unsharded) inputs -> full [B, N, DQ] output, on 8 NeuronCores."""
    from concourse.bass_utils import run_bass_kernel_spmd

    nc = _get_nc()
    in_maps = prep_all_inputs(**inputs)
    res = run_bass_kernel_spmd(nc, in_maps, list(range(NCORES)))
    return gather_outputs(res.results)


# revision 37
# speedup vs baseline: 1.1976x; 1.1976x over previous
"""Builder + host-side prep for nn_Attention distributed kernel.

Strategy: pure data-parallel sharding over (batch, query-row-half).
Core c handles batch b=c//2, query rows i0=(c%2)*512 .. i0+512.
No collectives needed: K/V are computed per-core from the full sequence
of its batch; each core's output rows are disjoint.

All attention math is done in "transposed score" layout S^T[j, i]
(j = key position on partitions, i = query on free axis) so the
probability matrix lands PV-ready without any on-chip transposes:
  - q^T, k^T projections: [e, n] layout from x^T (host pre-transposed)
  - S^T  = matmul(lhsT=k^T slice [d,j], rhs=q^T slice [d,i])
  - P^T  = exp(S^T) * exp(bias^T)   (host precomputes exp of bias;
    multiply on DVE).
  - out^T_h [33, i] = matmul(lhsT=v_aug [j, 33], rhs=P^T [j, i]) summed
    over j-chunks; column 32 of v_aug is ones -> row 32 = softmax denom.
    The two heads of a pair share one PSUM bank (partition offsets
    0/64) which lets their PV matmuls co-execute in the PE array.
  - PV batches execute two rounds behind QK batches (dep-edge stagger)
    so the exp->multiply chain is never on the QK critical path and the
    scalar engine streams exps back-to-back.
  - gating sigmoid via tanh (same ACT table set as exp); e-chunk 0's
    normalization broadcast via DMA-bounce mid-stream; chunk 1 via an
    indicator matmul at the tail (PE idle there), with the last head
    pair's PSUM evacuated on the scalar engine and the y-projection's
    first contraction half hoisted ahead of the normalization chain.
"""

import sys

if "/opt/trn_rl_repo" not in sys.path:
    sys.path.insert(0, "/opt/trn_rl_repo")

from contextlib import ExitStack

import ml_dtypes
import numpy as np

import concourse.bass as bass
import concourse.tile as tile
from concourse import bacc, mybir
from concourse.bass import ts

P = 128
B, N, DQ = 4, 1024, 256
H, D = 8, 32
NI = 512  # query rows per core
NCORES = 8

BF16 = mybir.dt.bfloat16
F32 = mybir.dt.float32
NPBF16 = ml_dtypes.bfloat16

EXPF = mybir.ActivationFunctionType.Exp
TANHF = mybir.ActivationFunctionType.Tanh
DIV = mybir.AluOpType.divide
COPYF = mybir.ActivationFunctionType.Copy



def build_nc():
    nc = bacc.Bacc(None, target_bir_lowering=False, debug=False)

    # DRAM parameters (identical graph on all 8 cores; shards differ)
    # exp(bias^T + mask), pre-tiled so each (h, jp) DMA is contiguous:
    # eb[h, jp, p, t*NI + i] = exp(bias^T)[h, (jp*2+t)*128 + p, i]
    eb_d = nc.declare_dram_parameter("eb", [H, 4, P, 2 * NI], BF16, False)
    # all bf16 weights + x packed per-partition: one DMA loads everything
    # layout per p: wq(2*256) wk wv wg wo | xt(2*1024) xqt(2*512)
    wpack_d = nc.declare_dram_parameter("wpack", [P, 5632], BF16, False)
    hbg_d = nc.declare_dram_parameter("hbg", [P, 2], F32, False)   # bg/2 as [p, chunk]
    bob_d = nc.declare_dram_parameter("bob", [P, 2], F32, False)   # bo as [p, c_chunk]
    ind_d = nc.declare_dram_parameter("ind", [8, 256], BF16, False)  # (e//32 == h)
    y_d = nc.declare_dram_parameter("out", [2, P, NI], BF16, True)  # y^T chunked
    dnrec_d = nc.dram_tensor("dnrec", [8, NI], F32)                # internal scratch

    from concourse.tile_rust import add_dep_helper

    with tile.TileContext(nc) as tc, ExitStack() as ctx:
        singles = ctx.enter_context(tc.tile_pool(name="singles", bufs=1))
        spsum = ctx.enter_context(tc.tile_pool(name="spsum", bufs=3, space="PSUM"))
        opsum = ctx.enter_context(tc.tile_pool(name="opsum", bufs=2, space="PSUM"))
        ebuf = ctx.enter_context(tc.tile_pool(name="ebuf", bufs=6))
        esb = ctx.enter_context(tc.tile_pool(name="esb", bufs=4))
        ptb = ctx.enter_context(tc.tile_pool(name="ptb", bufs=4))
        ostb = ctx.enter_context(tc.tile_pool(name="ostb", bufs=2))

        _sc = [0]

        def sslot():
            _sc[0] += 1
            return spsum.tile([P, 1024], F32, tag="s_ps", name=f"s_ps{_sc[0]}")

        # ---- load constants / weights (two packed DMAs on two queues) ----
        wpA = singles.tile([P, 4096], BF16, name="wpA")  # wq wk xt xqt
        wpB = singles.tile([P, 1536], BF16, name="wpB")  # wv wg wo
        nc.sync.dma_start(out=wpA, in_=wpack_d[:, 0:4096])
        nc.scalar.dma_start(out=wpB, in_=wpack_d[:, 4096:5632])
        wq_sb = wpA[:, 0:512].rearrange("p (k e) -> p k e", k=2)
        wk_sb = wpA[:, 512:1024].rearrange("p (k e) -> p k e", k=2)
        xt_sb = wpA[:, 1024:3072].rearrange("p (k n) -> p k n", k=2)
        xqt_sb = wpA[:, 3072:4096].rearrange("p (k i) -> p k i", k=2)
        wv_sb = wpB[:, 0:512].rearrange("p (k e) -> p k e", k=2)
        wg_sb = wpB[:, 512:1024].rearrange("p (k e) -> p k e", k=2)
        wo_sb = wpB[:, 1024:1536].rearrange("p (k e) -> p k e", k=2)
        hbg_sb = singles.tile([P, 2], F32)
        bob_sb = singles.tile([P, 2], F32)
        ind_sb = singles.tile([8, 256], BF16)
        nc.sync.dma_start(out=hbg_sb, in_=hbg_d[:])
        nc.sync.dma_start(out=bob_sb, in_=bob_d[:])
        nc.sync.dma_start(out=ind_sb, in_=ind_d[:])

        # ACT table preload: dummy Exp at t=0 so the 2.7us table load
        # overlaps the weight DMA instead of stalling the first real use
        warm = singles.tile([P, 8], F32)
        nc.vector.memset(warm, 1.0)
        nc.scalar.activation(out=warm, in_=warm, func=EXPF)

        # ---- projection targets ----
        kt_sb = [singles.tile([P, N], BF16, name=f"kt{m}") for m in range(2)]
        qt_sb = [singles.tile([P, NI], BF16, name=f"qt{m}") for m in range(2)]
        vaug_sb = [singles.tile([P, 2, H, 33], BF16, name=f"vaug{j}")
                   for j in range(4)]
        sig_sb = singles.tile([P, 2, NI], F32)    # sigmoid(gates)^T [e, i]
        ogt_un = singles.tile([P, 2, NI], F32)    # unnormalized gated^T staging

        def v_round(jtp):
            ps = sslot()
            f = l = None
            for u in range(2):
                jt = jtp * 2 + u
                for kc in range(2):
                    l = nc.tensor.matmul(
                        ps[:, u * 512 : u * 512 + 256],
                        lhsT=xt_sb[:, kc, ts(jt, P)], rhs=wv_sb[:, kc, :],
                        start=(kc == 0), stop=(kc == 1),
                    )
                    f = f or l
            for u in range(2):
                nc.vector.tensor_copy(
                    out=vaug_sb[jtp][:, u, :, 0:32],
                    in_=ps[:, u * 512 : u * 512 + 256].rearrange(
                        "p (h d) -> p h d", h=H),
                )
            return f, l

        def qk_round(m, part):
            # part 0: q chunk m + k chunk m first half; part 1: k second half
            ps = sslot()
            f = l = None
            if part == 0:
                for kc in range(2):
                    l = nc.tensor.matmul(
                        ps[:, :NI], lhsT=wq_sb[:, kc, ts(m, P)],
                        rhs=xqt_sb[:, kc, :], start=(kc == 0), stop=(kc == 1))
                    f = f or l
                for kc in range(2):
                    l = nc.tensor.matmul(
                        ps[:, NI:], lhsT=wk_sb[:, kc, ts(m, P)],
                        rhs=xt_sb[:, kc, :512], start=(kc == 0), stop=(kc == 1))
                nc.vector.tensor_copy(out=qt_sb[m], in_=ps[:, :NI])
                nc.vector.tensor_copy(out=kt_sb[m][:, 0:512], in_=ps[:, NI:])
            else:
                for kc in range(2):
                    l = nc.tensor.matmul(
                        ps[:, :NI], lhsT=wk_sb[:, kc, ts(m, P)],
                        rhs=xt_sb[:, kc, 512:], start=(kc == 0), stop=(kc == 1))
                    f = f or l
                nc.vector.tensor_copy(out=kt_sb[m][:, 512:], in_=ps[:, :NI])
            return f, l

        def g_round():
            # gates^T: sigmoid via tanh: sig = 0.5*tanh((g+bg)/2) + 0.5
            ps = sslot()
            f = l = None
            for m in range(2):
                for kc in range(2):
                    l = nc.tensor.matmul(
                        ps[:, ts(m, NI)], lhsT=wg_sb[:, kc, ts(m, P)],
                        rhs=xqt_sb[:, kc, :], start=(kc == 0), stop=(kc == 1))
                    f = f or l
            for m in range(2):
                nc.scalar.activation(out=sig_sb[:, m, :], in_=ps[:, ts(m, NI)],
                                     func=TANHF, bias=hbg_sb[:, m : m + 1],
                                     scale=0.5)
            nc.vector.tensor_scalar(out=sig_sb, in0=sig_sb, scalar1=0.5,
                                    scalar2=0.5, op0=mybir.AluOpType.mult,
                                    op1=mybir.AluOpType.add)
            return f, l

        # pre-stream: q/k chunk 0 + first v pair (everything else is injected
        # into the stream between QK and PV batches)
        qk_round(0, 0)
        for j in range(4):
            nc.vector.memset(vaug_sb[j][:, :, :, 32:33], 1.0)

        # injected work, keyed by stream round; PV emission lags two rounds
        # so each vaug tile only needs to be emitted one round before its
        # first PV consumer
        inject = {
            0: [lambda: qk_round(0, 1)],
            1: [lambda: v_round(0)],
            2: [lambda: v_round(1)],
            3: [lambda: v_round(2)],
            4: [lambda: v_round(3)],
            5: [lambda: qk_round(1, 0)],
            6: [lambda: qk_round(1, 1)],
            7: [g_round],
        }

        # ---- attention stream state ----
        dn8 = singles.tile([8, NI], F32)        # per-head denominators
        nc.vector.memset(dn8, 1.0)              # rows read before all written
        sigf = singles.tile([P, 2, NI], F32)    # sig * (1/denom broadcast)
        ogt = singles.tile([P, 2, NI], BF16)    # normalized gated out^T

        qk_insts, pv_insts, inj_insts = [], [], []

        def chunk_bounce(hc, rows, dmae):
            # reciprocal of denominators; broadcast each head's row to its
            # 32 partitions via a DRAM bounce with a stride-0 AP (no compute
            # engine in the path -> no PE queue blocking).  `rows` selects
            # which 32-partition groups of chunk hc to produce.
            rec = ostb.tile([8, NI], F32, tag="rec", name=f"rec{hc}_{rows[0]}")
            nc.vector.reciprocal_approx_fast(out=rec, in_=dn8)
            dmae.dma_start(out=dnrec_d[:], in_=rec)
            rb = ostb.tile([P, NI], F32, tag="rb", name=f"rb{hc}_{rows[0]}")
            for g in rows:
                sl = dnrec_d[hc * 4 + g : hc * 4 + g + 1, :]
                bcast_ap = bass.AP(tensor=sl.tensor, offset=sl.offset,
                                   ap=[[0, 32], list(sl.ap[1])])
                dmae.dma_start(out=rb[g * 32 : (g + 1) * 32, :], in_=bcast_ap)
            lo, hi = rows[0] * 32, (rows[-1] + 1) * 32
            nc.vector.tensor_mul(out=sigf[lo:hi, hc, :],
                                 in0=sig_sb[lo:hi, hc, :], in1=rb[lo:hi, :])
            nc.vector.tensor_mul(out=ogt[lo:hi, hc, :],
                                 in0=ogt_un[lo:hi, hc, :],
                                 in1=sigf[lo:hi, hc, :])

        o_tiles, pt_tiles = {}, {}

        def emit_pv(r):
            hp, jp = divmod(r, 4)
            if hp not in o_tiles:
                o_tiles[hp] = opsum.tile([P, NI], F32, tag="o",
                                         name=f"o_t{hp}")
            o_t = o_tiles[hp]
            pt = pt_tiles.pop(r)
            pf = pl = None
            for t in range(2):
                jc = jp * 2 + t
                for u, h in enumerate((2 * hp, 2 * hp + 1)):
                    pl = nc.tensor.matmul(
                        o_t[64 * u : 64 * u + 33, :],
                        lhsT=vaug_sb[jc // 2][:, jc % 2, h, :],
                        rhs=pt[h][:, t, :],
                        start=(jc == 0), stop=(jc == 7),
                        skip_group_check=True,
                    )
                    pf = pf or pl
            return pf, pl

        def emit_dumps(hp):
            o_t = o_tiles.pop(hp)
            for u, h in enumerate((2 * hp, 2 * hp + 1)):
                hq, hc = h % 4, h // 4
                ost = ostb.tile([33, NI], F32)
                if hp == 3:
                    # scalar engine is idle after the last exp
                    nc.scalar.activation(out=ost, in_=o_t[64 * u : 64 * u + 33, :],
                                         func=COPYF)
                    ea = nc.sync if h % 2 else nc.scalar
                    eb_ = nc.scalar if h % 2 else nc.sync
                else:
                    nc.vector.tensor_copy(out=ost,
                                          in_=o_t[64 * u : 64 * u + 33, :])
                    ea = eb_ = nc.gpsimd
                ea.dma_start(
                    out=ogt_un[hq * 32 : (hq + 1) * 32, hc, :], in_=ost[0:32, :]
                )
                eb_.dma_start(out=dn8[h : h + 1, :], in_=ost[32:33, :])

        for rnd in range(16):
            hpair, jp = divmod(rnd, 4)
            heads = (2 * hpair, 2 * hpair + 1)
            ebt, s_ps, pt = {}, {}, {}
            for h in heads:
                ebt[h] = ebuf.tile([P, 2, NI], BF16, tag="ebt",
                                   name=f"ebt{h}_{jp}")
                dq = nc.sync if h % 2 == 0 else nc.gpsimd
                dq.dma_start(out=ebt[h], in_=eb_d[h, jp])
            # interleaved QK batch: consecutive MMs hit different PE row
            # groups -> concurrent in the array
            for h in heads:
                s_ps[h] = sslot()
            qf = ql = None
            for t in range(2):
                jc = jp * 2 + t
                for h in heads:
                    hq, hc = h % 4, h // 4
                    ql = nc.tensor.matmul(
                        s_ps[h][:, ts(t, 512)],
                        lhsT=kt_sb[hc][hq * 32 : (hq + 1) * 32, ts(jc, P)],
                        rhs=qt_sb[hc][hq * 32 : (hq + 1) * 32, :],
                        start=True, stop=True,
                        tile_position=(hq * 32, 0),
                        skip_group_check=True,
                    )
                    qf = qf or ql
            qk_insts.append((qf, ql))
            # injected K=128 projection work sits between the QK and PV
            # batches (same array mode as PV -> no extra mode switches)
            ij = []
            for fn in inject.get(rnd, []):
                ij.append(fn())
            inj_insts.append(ij)
            for h in heads:
                pt[h] = ptb.tile([P, 2, NI], BF16, tag="pt",
                                 name=f"pt{h}_{rnd}")
                es = esb.tile([P, 2, NI], BF16, tag="es", name=f"es{h}_{jp}")
                nc.scalar.activation(
                    out=es, in_=s_ps[h][:].rearrange("p (t i) -> p t i", t=2),
                    func=EXPF,
                )
                nc.vector.tensor_mul(out=pt[h], in0=es, in1=ebt[h])
            pt_tiles[rnd] = pt
            if rnd >= 2:
                pv_insts.append(emit_pv(rnd - 2))
            if rnd % 4 == 1 and rnd >= 5:
                emit_dumps(rnd // 4 - 1)
            if rnd == 9:
                chunk_bounce(0, [0, 1, 2, 3], nc.gpsimd)
        pv_insts.append(emit_pv(14))
        pv_insts.append(emit_pv(15))
        emit_dumps(3)

        # PE batch order (the PE runs throttled when K=32 tile-position MMs
        # interleave with K=128 ones): [QK][inj][PV(r-2)] per round, staggered
        # two deep so exp/multiply never gate the QK stream:
        for r in range(len(qk_insts)):
            if r >= 3:
                add_dep_helper(qk_insts[r][0].ins, pv_insts[r - 3][1].ins,
                               sync=False, reason="qk(r) after pv(r-3)")
            for f, l in inj_insts[r]:
                add_dep_helper(f.ins, qk_insts[r][1].ins, sync=False,
                               reason="inj after qk batch")
            if r + 2 < len(qk_insts):
                prev = (inj_insts[r + 2][-1][1] if inj_insts[r + 2]
                        else qk_insts[r + 2][1])
                add_dep_helper(pv_insts[r][0].ins, prev.ins, sync=False,
                               reason="pv(r) after qk/inj(r+2)")

        # ---- tail: y-projection ec=0 halves run as soon as the PE is
        # free (ogt chunk 0 completed mid-stream); then the chunk-1
        # normalization (indicator-matmul broadcast, heads 4-7 in one go),
        # then the closing ec=1 matmuls + scalar-engine biased copies ----
        yps = sslot()
        for cc in range(2):
            nc.tensor.matmul(
                yps[:, ts(cc, NI)], lhsT=wo_sb[:, 0, ts(cc, P)],
                rhs=ogt[:, 0, :], start=True, stop=False,
                skip_group_check=True,
            )
        rec = ostb.tile([8, NI], F32, tag="rec", name="rec1")
        nc.vector.reciprocal_approx_fast(out=rec, in_=dn8)
        recb = ostb.tile([8, NI], BF16, tag="recb", name="recb1")
        nc.vector.tensor_copy(out=recb, in_=rec)
        bps = opsum.tile([P, NI], F32, tag="o", name="bps1")
        nc.tensor.matmul(bps[:, :NI], lhsT=ind_sb[:, ts(1, P)],
                         rhs=recb, start=True, stop=True)
        # ogt_un*sig precomputed while the reciprocal/broadcast chain runs
        nc.vector.tensor_mul(out=sigf[:, 1, :], in0=ogt_un[:, 1, :],
                             in1=sig_sb[:, 1, :])
        nc.vector.tensor_mul(out=ogt[:, 1, :], in0=sigf[:, 1, :],
                             in1=bps[:, :NI])
        for cc in range(2):
            nc.tensor.matmul(
                yps[:, ts(cc, NI)], lhsT=wo_sb[:, 1, ts(cc, P)],
                rhs=ogt[:, 1, :], start=False, stop=True,
                skip_group_check=True,
            )
            ysb = ostb.tile([P, NI], BF16, tag="ysb", name=f"ysb{cc}")
            nc.vector.tensor_scalar_add(out=ysb, in0=yps[:, ts(cc, NI)],
                                        scalar1=bob_sb[:, cc : cc + 1])
            dq = nc.sync if cc == 0 else nc.scalar
            dq.dma_start(out=y_d[cc], in_=ysb)

    nc.compile()
    return nc


def prep_core_inputs(core, x, mask, attn_bias, Wq, Wkv, Wo, bo, Wg, bg):
    """Host-side shard + layout prep for one core. All numpy."""
    b, ih = core // 2, core % 2
    i0 = ih * NI
    scale = D ** -0.5

    xt = np.ascontiguousarray(x[b].T)  # [256, N]
    amask = np.where(mask[b] > 0, 0.0, -200.0).astype(np.float32)  # [N] over j
    bt = attn_bias[b, :, i0 : i0 + NI, :].transpose(0, 2, 1)  # [H, j, i]
    bt = bt + amask[None, :, None]
    eb = np.exp(bt).astype(NPBF16)
    # pre-tile for contiguous per-(h, jp) DMAs: [H, 4, 128, 2*NI]
    eb = np.ascontiguousarray(
        eb.reshape(H, 4, 2, P, NI).transpose(0, 1, 3, 2, 4).reshape(H, 4, P, 2 * NI)
    )

    def chunk(wT):  # [256, X] -> [2, 128, X] bf16
        return np.ascontiguousarray(wT.reshape(2, P, -1)).astype(NPBF16)

    ind = np.zeros((8, 256), np.float32)
    for h in range(H):
        ind[h, h * 32 : (h + 1) * 32] = 1.0


    def flat(wT):  # [256, X] -> [128, 2*X] per-partition pack
        c = chunk(wT)  # [2, 128, X]
        return c.transpose(1, 0, 2).reshape(P, -1)

    wpack = np.concatenate(
        [flat((Wq * scale).T), flat(Wkv[:256].T), flat(xt),
         flat(xt[:, i0 : i0 + NI]), flat(Wkv[256:].T), flat(Wg.T),
         flat(Wo.T)], axis=1)
    return {
        "wpack": np.ascontiguousarray(wpack),
        "eb": np.ascontiguousarray(eb),
        "hbg": np.ascontiguousarray((bg * 0.5).reshape(2, P).T).astype(np.float32),
        "bob": np.ascontiguousarray(bo.astype(np.float32).reshape(2, P).T),
        "ind": ind.astype(NPBF16),
    }


def prep_all_inputs(**inputs):
    inputs = {k: np.asarray(v, dtype=np.float32) for k, v in inputs.items()}
    return [prep_core_inputs(c, **inputs) for c in range(NCORES)]


def gather_outputs(results):
    """results: per-core dicts with 'out' = y^T chunked [2, P, NI] -> [B, N, DQ]."""
    y = np.zeros((B, N, DQ), np.float32)
    for c in range(NCORES):
        b, ih = c // 2, c % 2
        yt = np.asarray(results[c]["out"]).astype(np.float32).reshape(DQ, NI)
        y[b, ih * NI : (ih + 1) * NI, :] = yt.T
    return y


_NC_CACHE = None


def _get_nc():
    global _NC_CACHE
    if _NC_CACHE is None:
        _NC_CACHE = build_nc()
    return _NC_CACHE


def kernel(**inputs):
    """Full (unsharded) inputs -> full [B, N, DQ] output, on 8 NeuronCores."""
    from concourse.bass_utils import run_bass_kernel_spmd

    nc = _get_nc()
    in_maps = prep_all_inputs(**inputs)
    res = run_bass_kernel_spmd(nc, in_maps, list(range(NCORES)))
    return gather_outputs(res.results)
